# revision 6
# baseline (speedup 1.0000x reference)
"""Trainium2 Bass kernel for nn_HardQuadTripletSOSRLoss.

Sharding: 8 cores = 2 batches x 4 HW-shards (4096 grid cells each).
Device job (per core): dsim scores = bf16(kp1_desc[b]) @ bf16(desc2f[b, shard])
-> fp32 PSUM (8 supertiles of [128 rows, 2048 cells]).  Each supertile is
consumed by one of two lanes:
  - 'direct': DVE max8 straight from PSUM -> top-8 cell values (fp32).
  - 'act':    ACT copies PSUM -> SBUF bf16, DVE tensor_max tree reduces
              2048 -> 512 (groups of 4 cells {o, o+512, o+1024, o+1536}),
              DVE max8 -> top-8 group maxima (bf16-precision values).
The two lanes run on different engines, halving the DVE-bound scan time.
Host merges candidates per row with a certificate (8th exported value per
window bounds anything hidden) and repairs rare rows exactly.  All other
stages (sampling, geometry, masks, SOS negatives, loss) run on host.
"""

import numpy as np
import ml_dtypes

import concourse.bass as bass
import concourse.mybir as mybir
import concourse.tile as tile
from concourse import bacc
from concourse.bass_utils import run_bass_kernel_spmd

# ---- problem constants (hardcoded per contract) ----
B, N, C, H, W = 2, 512, 128, 128, 128
HW = H * W
GS = 8
NUM_NEG = 16
SOS_NEG = 8
MARGIN = 1.0
NSHARD = 4
SHW = HW // NSHARD          # 4096 cells per shard
WIN = 2048                  # cells per candidate window (one supertile)
NWIN = SHW // WIN           # 2 windows per shard
RT = N // 128               # 4 row tiles
CPB = 512                   # columns per matmul (one PSUM bank)
GRP = 4                     # cells per group in 'act' lane candidates
GSTRIDE = WIN // GRP        # 512: group members are {o, o+512, o+1024, o+1536}

# lane per supertile g = 2*t + w  (t = row tile, w = window)
LANES = ["act", "direct", "act", "act", "direct", "act", "act", "act"]

F32 = mybir.dt.float32
BF16 = mybir.dt.bfloat16
BF = ml_dtypes.bfloat16

_NC_CACHE = {}
LAST_RESULTS = None  # BassKernelResults of most recent device run (for test.py)


def _build_nc():
    nc = bacc.Bacc("TRN2", target_bir_lowering=False, debug=False, num_devices=8)

    lhsT = nc.dram_tensor("lhsT", [C, N], BF16, kind="ExternalInput")
    rhs = nc.dram_tensor("rhs", [C, SHW], BF16, kind="ExternalInput")
    cand = nc.dram_tensor("cand", [RT, 128, NWIN * 8], F32, kind="ExternalOutput")

    QW = SHW // 4  # 1024-wide DMA quarters on separate queues

    with tile.TileContext(nc) as tc:
        with (
            tc.tile_pool(name="const", bufs=1) as cpool,
            tc.tile_pool(name="cnd", bufs=2) as cndpool,
            tc.tile_pool(name="cp", bufs=2) as cppool,
            tc.tile_pool(name="tr1", bufs=2) as tr1pool,
            tc.tile_pool(name="tr2", bufs=2) as tr2pool,
            tc.tile_pool(name="psum", bufs=2, space="PSUM") as pspool,
        ):
            lhsT_sb = cpool.tile([C, N], BF16, tag="lhsT")
            nc.sync.dma_start(lhsT_sb[:], lhsT[:, :])
            rhs_sb = []
            qeng = [nc.sync, nc.scalar, nc.gpsimd, nc.sync]
            for q in range(4):
                tq = cpool.tile([C, QW], BF16, tag=f"rhs{q}")
                qeng[q].dma_start(tq[:], rhs[:, q * QW : (q + 1) * QW])
                rhs_sb.append(tq)

            for t in range(RT):
                cn = cndpool.tile([128, NWIN * 8], F32, tag="cn")
                for w in range(NWIN):
                    g = 2 * t + w
                    ps = pspool.tile([128, WIN], F32, tag="ps")
                    for c in range(WIN // CPB):
                        col = w * WIN + c * CPB
                        q, qoff = divmod(col, QW)
                        nc.tensor.matmul(
                            ps[:, c * CPB : (c + 1) * CPB],
                            lhsT_sb[:, t * 128 : (t + 1) * 128],
                            rhs_sb[q][:, qoff : qoff + CPB],
                            start=True,
                            stop=True,
                        )
                    out8 = cn[:, w * 8 : (w + 1) * 8]
                    if LANES[g] == "direct":
                        nc.vector.max(out8, ps[:])
                    else:
                        sb = cppool.tile([128, WIN], BF16, tag="cp")
                        nc.scalar.copy(sb[:], ps[:])
                        u = tr1pool.tile([128, WIN // 2], BF16, tag="u")
                        nc.vector.tensor_max(
                            u[:], sb[:, : WIN // 2], sb[:, WIN // 2 :]
                        )
                        v = tr2pool.tile([128, WIN // 4], BF16, tag="v")
                        nc.vector.tensor_max(
                            v[:], u[:, : WIN // 4], u[:, WIN // 4 :]
                        )
                        nc.vector.max(out8, v[:])
                nc.sync.dma_start(cand[t], cn[:])

    nc.compile()
    return nc


def _get_nc():
    if "nc" not in _NC_CACHE:
        _NC_CACHE["nc"] = _build_nc()
    return _NC_CACHE["nc"]


# ---------------- host-side helpers (all float32, mirror reference) ----------


def _sample_descriptors(desc2, kp):
    """Bilinear sample of desc2 (B,C,H,W) at image-space (y,x) kp, L2-normed."""
    b, c, h, w = desc2.shape
    f = np.float32
    y = np.clip(kp[..., 0] / f(GS) - f(0.5), f(0.0), f(h - 1.0)).astype(f)
    x = np.clip(kp[..., 1] / f(GS) - f(0.5), f(0.0), f(w - 1.0)).astype(f)
    y0 = np.clip(np.floor(y), 0, h - 2).astype(np.int64)
    x0 = np.clip(np.floor(x), 0, w - 2).astype(np.int64)
    wy = (y - y0.astype(f))[..., None]
    wx = (x - x0.astype(f))[..., None]
    dmap = desc2.transpose(0, 2, 3, 1).reshape(b, h * w, c)

    def g(yi, xi):
        idx = yi * w + xi
        return np.take_along_axis(dmap, idx[..., None], axis=1)

    v = (
        g(y0, x0) * (1 - wy) * (1 - wx)
        + g(y0, x0 + 1) * (1 - wy) * wx
        + g(y0 + 1, x0) * wy * (1 - wx)
        + g(y0 + 1, x0 + 1) * wy * wx
    )
    n = np.sqrt(np.sum(v * v, axis=-1, keepdims=True)).astype(f)
    return (v / (n + f(1e-8))).astype(f)


def _nearest4(pts):
    """Flat ids (..., 4) of the 4 nearest grid-cell centers, matching the
    reference's top_k over all HW cells (ties -> lower flat id)."""
    f = np.float32
    y = pts[..., 0]
    x = pts[..., 1]
    cy = np.clip(np.floor(y / f(GS)).astype(np.int64), 0, H - 1)
    cx = np.clip(np.floor(x / f(GS)).astype(np.int64), 0, W - 1)
    by = np.clip(cy - 2, 0, H - 5)
    bx = np.clip(cx - 2, 0, W - 5)
    offs = np.arange(5, dtype=np.int64)
    iy = by[..., None] + offs          # (..., 5)
    ix = bx[..., None] + offs
    cyc = (f(GS) * iy + f(GS / 2.0)).astype(f)
    cxc = (f(GS) * ix + f(GS / 2.0)).astype(f)
    dy = y[..., None] - cyc
    dx = x[..., None] - cxc
    d2 = (dy * dy)[..., :, None] + (dx * dx)[..., None, :]   # (..., 5, 5)
    ids = iy[..., :, None] * W + ix[..., None, :]
    d2 = d2.reshape(d2.shape[:-2] + (25,))
    ids = ids.reshape(ids.shape[:-2] + (25,))
    # candidates are flat-id ascending, so a stable sort on d2 reproduces
    # top_k's lower-index tie-break
    order = np.argsort(d2, axis=-1, kind="stable")[..., :4]
    return np.take_along_axis(ids, order, axis=-1)


def _warp(p, Hm):
    f = np.float32
    xy = p[..., ::-1]
    ph = np.concatenate([xy, np.ones_like(xy[..., :1])], axis=-1)
    wp = np.einsum("bij,bmj->bmi", Hm, ph).astype(f)
    wp = wp[..., :2] / (wp[..., 2:3] + f(1e-8))
    return wp[..., ::-1].astype(f)


def _centers(ids):
    f = np.float32
    yy = (ids // W).astype(f) * f(GS) + f(GS / 2.0)
    xx = (ids % W).astype(f) * f(GS) + f(GS / 2.0)
    return np.stack([yy, xx], axis=-1)


def _smallest8_ids(sim):
    """Indices of the 8 smallest values per row of sim (B,N,N), reference
    tie-break (lower index wins)."""
    part = np.argpartition(sim, SOS_NEG + 1, axis=-1)[..., : SOS_NEG + 2]
    vals = np.take_along_axis(sim, part, axis=-1)
    order = np.lexsort((part, vals), axis=-1)[..., :SOS_NEG]
    return np.take_along_axis(part, order, axis=-1)


def kernel(kp1, w_kp1, kp1_desc, desc2, homo12):
    global LAST_RESULTS
    import os

    f = np.float32
    kp1 = np.asarray(kp1, f)
    w_kp1 = np.asarray(w_kp1, f)
    kp1_desc = np.asarray(kp1_desc, f)
    desc2 = np.asarray(desc2, f)
    homo12 = np.asarray(homo12, f)

    # ---------------- host geometry / small tensors ----------------
    w_kp1_desc = _sample_descriptors(desc2, w_kp1)                  # (B,N,C)
    pos = f(2.0) - f(2.0) * np.einsum("bnc,bnc->bn", kp1_desc, w_kp1_desc)

    cell4 = _nearest4(kp1)                                          # (B,N,4)
    kp1_cells = _centers(cell4.reshape(B, 4 * N))                   # (B,4N,2)
    warped = _warp(kp1_cells, homo12)                               # (B,4N,2)
    wcc = _nearest4(warped)                                         # (B,4N,4)
    ids16 = wcc.reshape(B, N, 16)                                   # neigh cells
    cell4_w = _nearest4(w_kp1)                                      # (B,N,4)

    # kp1_mask[n,n'] = #coinciding cells between cell4[n] and cell4[n']
    eqk = cell4[:, :, :, None, None] == cell4[:, None, None, :, :]
    kp1_mask = eqk.sum(axis=(2, 4)).astype(f)                       # (B,N,N)
    # w_kp1_mask[n,n'] = #coincidences between ids16[n] and cell4_w[n']
    eqw = ids16[:, :, :, None, None] == cell4_w[:, None, None, :, :]
    w_kp1_mask = eqw.sum(axis=(2, 4)).astype(f)                     # (B,N,N)

    # ---------------- sos (entirely host) ----------------
    k_sim = (f(2.0) - f(2.0) * np.einsum("bnc,bmc->bnm", kp1_desc, kp1_desc)
             + kp1_mask * f(5.0))
    w_sim = (f(2.0) - f(2.0) * np.einsum("bnc,bmc->bnm", w_kp1_desc, w_kp1_desc)
             + w_kp1_mask * f(5.0))
    k_ids = _smallest8_ids(k_sim)                                   # (B,N,8)
    w_ids = _smallest8_ids(w_sim)
    kd = np.take_along_axis(
        kp1_desc, k_ids.reshape(B, N * 8)[:, :, None], axis=1
    ).reshape(B, N, 8, C)
    wd = np.take_along_axis(
        w_kp1_desc, w_ids.reshape(B, N * 8)[:, :, None], axis=1
    ).reshape(B, N, 8, C)
    a = f(2.0) - f(2.0) * np.einsum("bnc,bnkc->bnk", kp1_desc, kd)
    bb = f(2.0) - f(2.0) * np.einsum("bnc,bnkc->bnk", w_kp1_desc, wd)
    sv = (a - bb).astype(f)
    sos = np.mean(np.sqrt(np.sum(sv * sv, axis=-1))).astype(f)

    # ---------------- device run: dsim candidate values ----------------
    nc = _get_nc()
    desc2_flat = desc2.reshape(B, C, HW)
    kp1_desc_bf = kp1_desc.astype(BF)
    desc2_bf = desc2_flat.astype(BF)
    in_maps = []
    for b in range(B):
        lhsT_b = np.ascontiguousarray(kp1_desc_bf[b].T)
        for s in range(NSHARD):
            in_maps.append(
                {
                    "lhsT": lhsT_b,
                    "rhs": np.ascontiguousarray(
                        desc2_bf[b][:, s * SHW : (s + 1) * SHW]
                    ),
                }
            )
    want_trace = bool(int(os.environ.get("KT_TRACE", "0")))
    try:
        res = run_bass_kernel_spmd(
            nc, in_maps, core_ids=list(range(8)), trace=want_trace
        )
    except ModuleNotFoundError:
        res = run_bass_kernel_spmd(nc, in_maps, core_ids=list(range(8)), trace=False)
    LAST_RESULTS = res
    results = res.results

    NW_TOT = NSHARD * NWIN                                          # 8 windows
    cand_all = np.empty((B, N, NW_TOT, 8), f)
    for ci, (b, s) in enumerate((b, s) for b in range(B) for s in range(NSHARD)):
        cnd = results[ci]["cand"]                                   # (RT,128,16)
        for t in range(RT):
            cand_all[b, t * 128 : (t + 1) * 128, s * NWIN : (s + 1) * NWIN] = (
                cnd[t].reshape(128, NWIN, 8)
            )

    # ---------------- fos: merge per-window candidates ----------------
    # Masked (neighbor) cells get +5 dsim in the reference => can never be
    # in the true top-16; identify their exported candidates by value match
    # and drop them (re-inserting the best unmasked group sibling for 'act'
    # windows).  Certificate: any window whose 8th exported value could
    # reach the merged top-16 triggers an exact host repair of that row.
    q_bf = kp1_desc_bf.astype(f)                                    # (B,N,C)
    d_bf = desc2_bf.astype(f).transpose(0, 2, 1)                    # (B,HW,C)

    # device-precision scores of masked cells and their 'act' group siblings
    o_in_win = ids16 % WIN                                          # (B,N,16)
    gbase = ids16 - o_in_win + (o_in_win % GSTRIDE)                 # group base id
    sib = gbase[..., None] + (np.arange(GRP) * GSTRIDE)             # (B,N,16,4)
    gath = np.take_along_axis(
        d_bf, sib.reshape(B, N * 16 * GRP)[:, :, None], axis=1
    ).reshape(B, N, 16, GRP, C)
    vsib = np.einsum("bnc,bnjgc->bnjg", q_bf, gath).astype(f)       # fp32 dots
    vsib_bf = vsib.astype(BF).astype(f)                             # act-lane vals

    win_min = cand_all[..., 7]                                      # (B,N,8)
    widx_of = ids16 // WIN                                          # global window
    member_of = o_in_win // GSTRIDE                                 # group member
    MATCH_TOL = 1e-4
    hwdesc = desc2_flat.transpose(0, 2, 1)                          # (B,HW,C) f32

    neg_scores = np.empty((B, N, NUM_NEG), f)
    repair = []
    for b in range(B):
        for n in range(N):
            t = n // 128
            cv = cand_all[b, n].copy()                              # (8, 8) desc
            extra = []
            ok = True
            seen = set()
            cells = ids16[b, n]
            for m in range(16):
                cell = int(cells[m])
                if cell in seen:
                    continue
                seen.add(cell)
                widx = int(widx_of[b, n, m])
                lane = LANES[2 * t + (widx % NWIN)]
                me = int(member_of[b, n, m])
                if lane == "direct":
                    v = vsib[b, n, m, me]                           # exact fp32
                    if v < win_min[b, n, widx] - MATCH_TOL:
                        continue                                    # not exported
                else:
                    vals = vsib_bf[b, n, m]                         # (4,) group
                    if vals[me] < vals.max():
                        continue                                    # sibling won
                    v = vals[me]
                    if v < win_min[b, n, widx] - MATCH_TOL:
                        continue                                    # not exported
                d = np.abs(cv[widx] - v)
                hits = np.nonzero(d <= MATCH_TOL)[0]
                if len(hits) != 1:
                    ok = False                                      # ambiguous
                    break
                cv[widx, hits[0]] = -np.inf
                if lane == "act":
                    # re-insert the best unmasked sibling of this group
                    masked_members = {
                        int(member_of[b, n, j])
                        for j in range(16)
                        if int(gbase[b, n, j]) == int(gbase[b, n, m])
                    }
                    unmasked = [
                        vals[k] for k in range(GRP) if k not in masked_members
                    ]
                    if unmasked:
                        extra.append(max(unmasked))
            if not ok:
                repair.append((b, n))
                continue
            merged = np.sort(np.concatenate([cv.reshape(-1), np.array(extra, f)]))[
                ::-1
            ]
            t16 = merged[NUM_NEG - 1]
            if (win_min[b, n] >= t16 - 1e-6).any():
                repair.append((b, n))
                continue
            neg_scores[b, n] = merged[:NUM_NEG]

    for b, n in repair:
        row = hwdesc[b] @ kp1_desc[b, n]                            # (HW,) f32
        row[ids16[b, n]] = -np.inf
        neg_scores[b, n] = np.sort(row)[-NUM_NEG:][::-1]

    neg = f(2.0) - f(2.0) * neg_scores                              # (B,N,16)
    fos = np.mean(
        np.maximum(pos[..., None] - neg + f(MARGIN), f(0.0)) ** 2
    ).astype(f)

    return np.asarray(fos + sos, dtype=np.float32)


# revision 10
# speedup vs baseline: 1.0224x; 1.0224x over previous
"""Trainium2 Bass kernel for nn_HardQuadTripletSOSRLoss.

Sharding: 8 cores = 2 batches x 4 HW-shards (4096 grid cells each).
Device job (per core): dsim scores = bf16(kp1_desc[b]) @ bf16(desc2f[b, shard])
-> fp32 PSUM (8 supertiles of [128 rows, 2048 cells]).  Each supertile is
consumed by one of two lanes:
  - 'direct': DVE max8 straight from PSUM -> top-8 cell values (fp32).
  - 'act':    ACT copies PSUM -> SBUF bf16, DVE tensor_max tree reduces
              2048 -> 512 (groups of 4 cells {o, o+512, o+1024, o+1536}),
              DVE max8 -> top-8 group maxima (bf16-precision values).
The two lanes run on different engines, halving the DVE-bound scan time.
Host merges candidates per row with a certificate (8th exported value per
window bounds anything hidden) and repairs rare rows exactly.  All other
stages (sampling, geometry, masks, SOS negatives, loss) run on host.
"""

import numpy as np
import ml_dtypes

import concourse.bass as bass
import concourse.mybir as mybir
import concourse.tile as tile
from concourse import bacc
from concourse.bass_utils import run_bass_kernel_spmd

# ---- problem constants (hardcoded per contract) ----
B, N, C, H, W = 2, 512, 128, 128, 128
HW = H * W
GS = 8
NUM_NEG = 16
SOS_NEG = 8
MARGIN = 1.0
NSHARD = 4
SHW = HW // NSHARD          # 4096 cells per shard
WIN = 2048                  # cells per candidate window (one supertile)
NWIN = SHW // WIN           # 2 windows per shard
RT = N // 128               # 4 row tiles
CPB = 512                   # columns per matmul (one PSUM bank)
GRP = 4                     # cells per group in 'act' lane candidates
GSTRIDE = WIN // GRP        # 512: group members are {o, o+512, o+1024, o+1536}

# lane per supertile g = 2*t + w  (t = row tile, w = window)
LANES = ["direct"] * 8

F32 = mybir.dt.float32
BF16 = mybir.dt.bfloat16
BF = ml_dtypes.bfloat16

_NC_CACHE = {}
LAST_RESULTS = None  # BassKernelResults of most recent device run (for test.py)


def _build_nc():
    nc = bacc.Bacc("TRN2", target_bir_lowering=False, debug=False, num_devices=8)

    lhsT = nc.dram_tensor("lhsT", [C, N], BF16, kind="ExternalInput")
    rhs = nc.dram_tensor("rhs", [C, SHW], BF16, kind="ExternalInput")
    # candidate outputs: [128 rows-within-tile, RT*NWIN*8]
    cand = nc.dram_tensor("cand", [128, RT * NWIN * 8], F32, kind="ExternalOutput")

    NSUP = RT * NWIN  # 8 supertiles, g = 2*t + w

    with (
        nc.sbuf_tensor([C, N], BF16) as lhsT_sb,
        nc.sbuf_tensor([C, SHW], BF16) as rhs_sb,
        nc.sbuf_tensor([128, NSUP * 8], F32) as cn_sb,
        nc.psum_tensor([128, 2 * WIN], F32) as ps,
        nc.semaphore() as dma_sem,
        nc.semaphore() as mm_sem,
        nc.semaphore() as dve_sem,
        nc.Block() as block,
    ):

        @block.sync
        def _(sync):
            sync.dma_start(lhsT_sb[:], lhsT[:, :]).then_inc(dma_sem, 16)
            sync.dma_start(rhs_sb[:, :WIN], rhs[:, :WIN]).then_inc(dma_sem, 16)
            sync.dma_start(
                rhs_sb[:, WIN : 2 * WIN], rhs[:, WIN : 2 * WIN]
            ).then_inc(dma_sem, 16)
            sync.wait_ge(dve_sem, NSUP)
            sync.dma_start(cand[:, :], cn_sb[:]).then_inc(dma_sem, 16)
            sync.wait_ge(dma_sem, 64)

        @block.tensor
        def _(tensor):
            for g in range(NSUP):
                t, w = divmod(g, NWIN)
                # inputs ready: lhsT + enough of rhs
                if g == 0:
                    tensor.wait_ge(dma_sem, 32)
                elif g == 1:
                    tensor.wait_ge(dma_sem, 48)
                # PSUM half g%2 free once DVE consumed supertile g-2
                if g >= 2:
                    tensor.wait_ge(dve_sem, g - 1)
                pso = (g % 2) * WIN
                for c in range(WIN // CPB):
                    mm = nc.tensor.matmul(
                        ps[:, pso + c * CPB : pso + (c + 1) * CPB],
                        lhsT_sb[:, t * 128 : (t + 1) * 128],
                        rhs_sb[:, w * WIN + c * CPB : w * WIN + (c + 1) * CPB],
                        start=True,
                        stop=True,
                    )
                    if c == WIN // CPB - 1:
                        mm.then_inc(mm_sem, 1)

        @block.vector
        def _(vector):
            for g in range(NSUP):
                vector.wait_ge(mm_sem, g + 1)
                pso = (g % 2) * WIN
                nc.vector.max(
                    cn_sb[:, g * 8 : (g + 1) * 8], ps[:, pso : pso + WIN]
                ).then_inc(dve_sem, 1)

    nc.compile()
    return nc


def _get_nc():
    if "nc" not in _NC_CACHE:
        _NC_CACHE["nc"] = _build_nc()
    return _NC_CACHE["nc"]


# ---------------- host-side helpers (all float32, mirror reference) ----------


def _sample_descriptors(desc2, kp):
    """Bilinear sample of desc2 (B,C,H,W) at image-space (y,x) kp, L2-normed."""
    b, c, h, w = desc2.shape
    f = np.float32
    y = np.clip(kp[..., 0] / f(GS) - f(0.5), f(0.0), f(h - 1.0)).astype(f)
    x = np.clip(kp[..., 1] / f(GS) - f(0.5), f(0.0), f(w - 1.0)).astype(f)
    y0 = np.clip(np.floor(y), 0, h - 2).astype(np.int64)
    x0 = np.clip(np.floor(x), 0, w - 2).astype(np.int64)
    wy = (y - y0.astype(f))[..., None]
    wx = (x - x0.astype(f))[..., None]
    dmap = desc2.transpose(0, 2, 3, 1).reshape(b, h * w, c)

    def g(yi, xi):
        idx = yi * w + xi
        return np.take_along_axis(dmap, idx[..., None], axis=1)

    v = (
        g(y0, x0) * (1 - wy) * (1 - wx)
        + g(y0, x0 + 1) * (1 - wy) * wx
        + g(y0 + 1, x0) * wy * (1 - wx)
        + g(y0 + 1, x0 + 1) * wy * wx
    )
    n = np.sqrt(np.sum(v * v, axis=-1, keepdims=True)).astype(f)
    return (v / (n + f(1e-8))).astype(f)


def _nearest4(pts):
    """Flat ids (..., 4) of the 4 nearest grid-cell centers, matching the
    reference's top_k over all HW cells (ties -> lower flat id)."""
    f = np.float32
    y = pts[..., 0]
    x = pts[..., 1]
    cy = np.clip(np.floor(y / f(GS)).astype(np.int64), 0, H - 1)
    cx = np.clip(np.floor(x / f(GS)).astype(np.int64), 0, W - 1)
    by = np.clip(cy - 2, 0, H - 5)
    bx = np.clip(cx - 2, 0, W - 5)
    offs = np.arange(5, dtype=np.int64)
    iy = by[..., None] + offs          # (..., 5)
    ix = bx[..., None] + offs
    cyc = (f(GS) * iy + f(GS / 2.0)).astype(f)
    cxc = (f(GS) * ix + f(GS / 2.0)).astype(f)
    dy = y[..., None] - cyc
    dx = x[..., None] - cxc
    d2 = (dy * dy)[..., :, None] + (dx * dx)[..., None, :]   # (..., 5, 5)
    ids = iy[..., :, None] * W + ix[..., None, :]
    d2 = d2.reshape(d2.shape[:-2] + (25,))
    ids = ids.reshape(ids.shape[:-2] + (25,))
    # candidates are flat-id ascending, so a stable sort on d2 reproduces
    # top_k's lower-index tie-break
    order = np.argsort(d2, axis=-1, kind="stable")[..., :4]
    return np.take_along_axis(ids, order, axis=-1)


def _warp(p, Hm):
    f = np.float32
    xy = p[..., ::-1]
    ph = np.concatenate([xy, np.ones_like(xy[..., :1])], axis=-1)
    wp = np.einsum("bij,bmj->bmi", Hm, ph).astype(f)
    wp = wp[..., :2] / (wp[..., 2:3] + f(1e-8))
    return wp[..., ::-1].astype(f)


def _centers(ids):
    f = np.float32
    yy = (ids // W).astype(f) * f(GS) + f(GS / 2.0)
    xx = (ids % W).astype(f) * f(GS) + f(GS / 2.0)
    return np.stack([yy, xx], axis=-1)


def _smallest8_ids(sim):
    """Indices of the 8 smallest values per row of sim (B,N,N), reference
    tie-break (lower index wins)."""
    part = np.argpartition(sim, SOS_NEG + 1, axis=-1)[..., : SOS_NEG + 2]
    vals = np.take_along_axis(sim, part, axis=-1)
    order = np.lexsort((part, vals), axis=-1)[..., :SOS_NEG]
    return np.take_along_axis(part, order, axis=-1)


def kernel(kp1, w_kp1, kp1_desc, desc2, homo12):
    global LAST_RESULTS
    import os

    f = np.float32
    kp1 = np.asarray(kp1, f)
    w_kp1 = np.asarray(w_kp1, f)
    kp1_desc = np.asarray(kp1_desc, f)
    desc2 = np.asarray(desc2, f)
    homo12 = np.asarray(homo12, f)

    # ---------------- host geometry / small tensors ----------------
    w_kp1_desc = _sample_descriptors(desc2, w_kp1)                  # (B,N,C)
    pos = f(2.0) - f(2.0) * np.einsum("bnc,bnc->bn", kp1_desc, w_kp1_desc)

    cell4 = _nearest4(kp1)                                          # (B,N,4)
    kp1_cells = _centers(cell4.reshape(B, 4 * N))                   # (B,4N,2)
    warped = _warp(kp1_cells, homo12)                               # (B,4N,2)
    wcc = _nearest4(warped)                                         # (B,4N,4)
    ids16 = wcc.reshape(B, N, 16)                                   # neigh cells
    cell4_w = _nearest4(w_kp1)                                      # (B,N,4)

    # kp1_mask[n,n'] = #coinciding cells between cell4[n] and cell4[n']
    eqk = cell4[:, :, :, None, None] == cell4[:, None, None, :, :]
    kp1_mask = eqk.sum(axis=(2, 4)).astype(f)                       # (B,N,N)
    # w_kp1_mask[n,n'] = #coincidences between ids16[n] and cell4_w[n']
    eqw = ids16[:, :, :, None, None] == cell4_w[:, None, None, :, :]
    w_kp1_mask = eqw.sum(axis=(2, 4)).astype(f)                     # (B,N,N)

    # ---------------- sos (entirely host) ----------------
    k_sim = (f(2.0) - f(2.0) * np.einsum("bnc,bmc->bnm", kp1_desc, kp1_desc)
             + kp1_mask * f(5.0))
    w_sim = (f(2.0) - f(2.0) * np.einsum("bnc,bmc->bnm", w_kp1_desc, w_kp1_desc)
             + w_kp1_mask * f(5.0))
    k_ids = _smallest8_ids(k_sim)                                   # (B,N,8)
    w_ids = _smallest8_ids(w_sim)
    kd = np.take_along_axis(
        kp1_desc, k_ids.reshape(B, N * 8)[:, :, None], axis=1
    ).reshape(B, N, 8, C)
    wd = np.take_along_axis(
        w_kp1_desc, w_ids.reshape(B, N * 8)[:, :, None], axis=1
    ).reshape(B, N, 8, C)
    a = f(2.0) - f(2.0) * np.einsum("bnc,bnkc->bnk", kp1_desc, kd)
    bb = f(2.0) - f(2.0) * np.einsum("bnc,bnkc->bnk", w_kp1_desc, wd)
    sv = (a - bb).astype(f)
    sos = np.mean(np.sqrt(np.sum(sv * sv, axis=-1))).astype(f)

    # ---------------- device run: dsim candidate values ----------------
    nc = _get_nc()
    desc2_flat = desc2.reshape(B, C, HW)
    kp1_desc_bf = kp1_desc.astype(BF)
    desc2_bf = desc2_flat.astype(BF)
    in_maps = []
    for b in range(B):
        lhsT_b = np.ascontiguousarray(kp1_desc_bf[b].T)
        for s in range(NSHARD):
            in_maps.append(
                {
                    "lhsT": lhsT_b,
                    "rhs": np.ascontiguousarray(
                        desc2_bf[b][:, s * SHW : (s + 1) * SHW]
                    ),
                }
            )
    want_trace = bool(int(os.environ.get("KT_TRACE", "0")))
    try:
        res = run_bass_kernel_spmd(
            nc, in_maps, core_ids=list(range(8)), trace=want_trace
        )
    except ModuleNotFoundError:
        res = run_bass_kernel_spmd(nc, in_maps, core_ids=list(range(8)), trace=False)
    LAST_RESULTS = res
    results = res.results

    NW_TOT = NSHARD * NWIN                                          # 8 windows
    cand_all = np.empty((B, N, NW_TOT, 8), f)
    for ci, (b, s) in enumerate((b, s) for b in range(B) for s in range(NSHARD)):
        cnd = results[ci]["cand"]                                   # (128, RT*16)
        for t in range(RT):
            cand_all[b, t * 128 : (t + 1) * 128, s * NWIN : (s + 1) * NWIN] = (
                cnd[:, t * NWIN * 8 : (t + 1) * NWIN * 8].reshape(128, NWIN, 8)
            )

    # ---------------- fos: merge per-window candidates ----------------
    # Masked (neighbor) cells get +5 dsim in the reference => can never be
    # in the true top-16; identify their exported candidates by value match
    # and drop them (re-inserting the best unmasked group sibling for 'act'
    # windows).  Certificate: any window whose 8th exported value could
    # reach the merged top-16 triggers an exact host repair of that row.
    q_bf = kp1_desc_bf.astype(f)                                    # (B,N,C)
    d_bf = desc2_bf.astype(f).transpose(0, 2, 1)                    # (B,HW,C)

    # device-precision scores of masked cells and their 'act' group siblings
    o_in_win = ids16 % WIN                                          # (B,N,16)
    gbase = ids16 - o_in_win + (o_in_win % GSTRIDE)                 # group base id
    sib = gbase[..., None] + (np.arange(GRP) * GSTRIDE)             # (B,N,16,4)
    gath = np.take_along_axis(
        d_bf, sib.reshape(B, N * 16 * GRP)[:, :, None], axis=1
    ).reshape(B, N, 16, GRP, C)
    vsib = np.einsum("bnc,bnjgc->bnjg", q_bf, gath).astype(f)       # fp32 dots
    vsib_bf = vsib.astype(BF).astype(f)                             # act-lane vals

    win_min = cand_all[..., 7]                                      # (B,N,8)
    widx_of = ids16 // WIN                                          # global window
    member_of = o_in_win // GSTRIDE                                 # group member
    MATCH_TOL = 1e-4
    hwdesc = desc2_flat.transpose(0, 2, 1)                          # (B,HW,C) f32

    neg_scores = np.empty((B, N, NUM_NEG), f)
    repair = []
    for b in range(B):
        for n in range(N):
            t = n // 128
            cv = cand_all[b, n].copy()                              # (8, 8) desc
            extra = []
            ok = True
            seen = set()
            cells = ids16[b, n]
            for m in range(16):
                cell = int(cells[m])
                if cell in seen:
                    continue
                seen.add(cell)
                widx = int(widx_of[b, n, m])
                lane = LANES[2 * t + (widx % NWIN)]
                me = int(member_of[b, n, m])
                if lane == "direct":
                    v = vsib[b, n, m, me]                           # exact fp32
                    if v < win_min[b, n, widx] - MATCH_TOL:
                        continue                                    # not exported
                else:
                    vals = vsib_bf[b, n, m]                         # (4,) group
                    if vals[me] < vals.max():
                        continue                                    # sibling won
                    v = vals[me]
                    if v < win_min[b, n, widx] - MATCH_TOL:
                        continue                                    # not exported
                d = np.abs(cv[widx] - v)
                hits = np.nonzero(d <= MATCH_TOL)[0]
                if len(hits) != 1:
                    ok = False                                      # ambiguous
                    break
                cv[widx, hits[0]] = -np.inf
                if lane == "act":
                    # re-insert the best unmasked sibling of this group
                    masked_members = {
                        int(member_of[b, n, j])
                        for j in range(16)
                        if int(gbase[b, n, j]) == int(gbase[b, n, m])
                    }
                    unmasked = [
                        vals[k] for k in range(GRP) if k not in masked_members
                    ]
                    if unmasked:
                        extra.append(max(unmasked))
            if not ok:
                repair.append((b, n))
                continue
            merged = np.sort(np.concatenate([cv.reshape(-1), np.array(extra, f)]))[
                ::-1
            ]
            t16 = merged[NUM_NEG - 1]
            if (win_min[b, n] >= t16 - 1e-6).any():
                repair.append((b, n))
                continue
            neg_scores[b, n] = merged[:NUM_NEG]

    for b, n in repair:
        row = hwdesc[b] @ kp1_desc[b, n]                            # (HW,) f32
        row[ids16[b, n]] = -np.inf
        neg_scores[b, n] = np.sort(row)[-NUM_NEG:][::-1]

    neg = f(2.0) - f(2.0) * neg_scores                              # (B,N,16)
    fos = np.mean(
        np.maximum(pos[..., None] - neg + f(MARGIN), f(0.0)) ** 2
    ).astype(f)

    return np.asarray(fos + sos, dtype=np.float32)
